# revision 1
# baseline (speedup 1.0000x reference)
"""TRN2 Bass kernel v4 for nn_AlignHead (deformable conv 3x3 + ReLU + 1x1 cls).

Baseline (v1) structure -- 4-parity TC in DRAM + per-slot indirect gathers,
position-major interp on DVE, PE transpose-back, PSUM conv accumulation --
with the schedule fixed:
  - ALL bilinear index/weight math precomputed on the HOST from the offset
    input (v1 burned ~250us of serial DVE at kernel start computing it).
  - TC zero-pad DMAs moved from gpsimd (SWDGE) to sync (HWDGE): the Pool
    engine only issues the 576 indirect gathers.
  - Deeper tile buffering for run-ahead.
  - Interp / transpose-back / conv matmuls split into slot-halves (sl 0-3,
    sl 4-7 == matmul nb halves): each half starts as soon as its 4 gathers
    land instead of waiting for all 8, shortening the per-tap serial tail.
Numerics identical to v1: f16 gather/matmul data, f32 PSUM accumulation.
"""

import sys
sys.path.insert(0, '/opt/trn_rl_repo')
import numpy as np

import concourse.bass as bass
import concourse.tile as tile
from concourse import bacc, mybir

f32 = mybir.dt.float32
f16 = mybir.dt.float16
i32 = mybir.dt.int32

N_CORES = 8
C = 256
H = W = 64
HW = H * W
NPB = 4             # position blocks per image
PB = HW // NPB      # 1024 positions per block
SL = PB // 128      # 8 slots (gathers) per block
G = 34              # tile grid per axis (m in [-1,32] stored at m+1)
NT = G * G          # tiles per parity copy
TCN = 4 * NT        # tiles per image (4 parity copies)
Ao = mybir.AluOpType
Act = mybir.ActivationFunctionType


def host_ident():
    return np.eye(128, dtype=np.float16)


def pack_weights(w_def, w_cls):
    w = np.asarray(w_def, dtype=np.float32).reshape(256, 256, 9)
    wp = np.empty((128, 9, 2, 2, 128), dtype=np.float32)
    for cc in range(2):
        for oc in range(2):
            blk = w[oc * 128:(oc + 1) * 128, cc * 128:(cc + 1) * 128, :]
            wp[:, :, cc, oc, :] = blk.transpose(1, 2, 0)
    wc = np.asarray(w_cls, dtype=np.float32).reshape(256)
    wcp = wc.reshape(2, 128).T.copy()
    return wp, wcp


def host_indices(off_img):
    """offsets [n_img, HW, 18] f32 -> (idx [128, n_img, 9, 4, 8] i32 TC tile
    ids, w4 [128, n_img, 9, 4, 8, 4, 2] f16 corner weights, dup'd in pairs).

    Matches v1's on-device math: tile id = (2a+b)*NT + (m+1)*G + (n+1) with
    a = y0 mod 2, m = clip(floor(y0/2), -1, 32) (same for b, n from x0).
    Position mapping: pos = pb*1024 + sl*128 + p.
    """
    n_img = off_img.shape[0]
    off = np.asarray(off_img, dtype=np.float32)
    pos = np.arange(HW, dtype=np.float32)
    ybase = np.floor(pos / W)
    xbase = pos - ybase * W
    idx_out = np.zeros((128, n_img, 9, NPB, SL), dtype=np.int32)
    w4_out = np.zeros((128, n_img, 9, NPB, SL, 4, 2), dtype=np.float16)
    for k in range(9):
        ky, kx = k // 3, k % 3
        py = (ybase + np.float32(ky - 1)) + off[:, :, 2 * k]
        px = (xbase + np.float32(kx - 1)) + off[:, :, 2 * k + 1]
        y0 = np.floor(py)
        x0 = np.floor(px)
        fy = py - y0
        fx = px - x0
        wy0 = (1.0 - fy) * ((y0 >= 0) & (y0 <= H - 1))
        wy1 = fy * ((y0 >= -1) & (y0 <= H - 2))
        wx0 = (1.0 - fx) * ((x0 >= 0) & (x0 <= W - 1))
        wx1 = fx * ((x0 >= -1) & (x0 <= W - 2))
        ma = np.floor(y0 * 0.5)
        a = y0 - 2.0 * ma
        m = np.clip(ma, -1, 32)
        nb = np.floor(x0 * 0.5)
        b = x0 - 2.0 * nb
        n = np.clip(nb, -1, 32)
        tid = ((2 * a + b) * NT + (m + 1) * G + (n + 1)).astype(np.int32)
        wgt = np.stack([wy0 * wx0, wy0 * wx1, wy1 * wx0, wy1 * wx1],
                       axis=-1).astype(np.float16)          # [n, HW, 4]
        for img in range(n_img):
            for pb in range(NPB):
                sl = slice(pb * PB, (pb + 1) * PB)
                idx_out[:, img, k, pb, :] = tid[img, sl].reshape(SL, 128).T
                wb = wgt[img, sl].reshape(SL, 128, 4)       # [sl, p, j]
                for d in range(2):
                    w4_out[:, img, k, pb, :, :, d] = wb.transpose(1, 0, 2)
    return idx_out, w4_out


def build(n_img, queues=True):
    nc = bacc.Bacc("TRN2", target_bir_lowering=False, debug=False,
                   num_devices=N_CORES, num_swdge_queues=4 if queues else 1)
    x_d = nc.dram_tensor("x", [n_img, C, HW], f32, kind="ExternalInput").ap()
    idx_d = nc.dram_tensor("idx", [128, n_img, 9, NPB, SL], i32,
                           kind="ExternalInput").ap()
    w4_d = nc.dram_tensor("w4", [128, n_img, 9, NPB, SL, 4, 2], f16,
                          kind="ExternalInput").ap()
    wdef_d = nc.dram_tensor("wdef", [128, 9, 2, 2, 128], f32, kind="ExternalInput").ap()
    wcls_d = nc.dram_tensor("wcls", [128, 2], f32, kind="ExternalInput").ap()
    bcls_d = nc.dram_tensor("bcls", [1, 1], f32, kind="ExternalInput").ap()
    ident_d = nc.dram_tensor("ident", [128, 128], f16, kind="ExternalInput").ap()
    out_d = nc.dram_tensor("out", [n_img, HW], f32, kind="ExternalOutput").ap()

    xT_d = nc.dram_tensor("xT", [n_img, HW, C], f16).ap()
    TC_ds = [nc.dram_tensor(f"TC{i}", [TCN, 4 * C], f16).ap()
             for i in range(n_img)]

    with tile.TileContext(nc) as tc:
        with tc.tile_pool(name="const", bufs=1) as constp:
            ident = constp.tile([128, 128], f16)
            nc.sync.dma_start(ident[:], ident_d[:])
            wdef = constp.tile([128, 9, 2, 2, 128], f16)
            with tc.tile_pool(name="wstage", bufs=1) as wsp:
                wst = wsp.tile([128, 9, 2, 2, 128], f32)
                nc.sync.dma_start(wst[:], wdef_d[:])
                nc.scalar.activation(wdef[:], wst[:], Act.Copy)
            wcls = constp.tile([128, 2], f16)
            wclsf = constp.tile([128, 2], f32)
            nc.sync.dma_start(wclsf[:], wcls_d[:])
            nc.vector.tensor_copy(wcls[:], wclsf[:])
            bcls = constp.tile([1, 1], f32)
            nc.sync.dma_start(bcls[:], bcls_d[:])
            zpad = constp.tile([128, 1024], f16)
            nc.vector.memset(zpad[:], 0.0)
            idx16 = constp.tile([128, n_img, 9, NPB, SL], i32)
            nc.sync.dma_start(idx16[:], idx_d[:])
            w4 = constp.tile([128, n_img, 9, NPB, SL, 4, 2], f16)
            nc.sync.dma_start(w4[:], w4_d[:])

            for img in range(n_img):
                # ---------- phase 1: xT + TC build ----------
                tcflat = TC_ds[img][:].rearrange("t e -> (t e)")
                E = 4 * C

                def zdma(dst_off, dims, zsrc):
                    nc.sync.dma_start(
                        bass.AP(tcflat.tensor, tcflat.offset + dst_off, dims),
                        zsrc)

                for a in range(2):
                    for b in range(2):
                        base = ((a * 2 + b) * NT) * E
                        if a == 0:
                            for R in (0, 33):
                                zdma(base + R * G * E, [[1024, 34], [1, 1024]],
                                     zpad[0:34, :])
                        else:
                            zdma(base + 33 * G * E, [[1024, 34], [1, 1024]],
                                 zpad[0:34, :])
                            zdma(base + 0 * G * E, [[1024, 34], [1, 512]],
                                 zpad[0:34, 0:512])
                            zdma(base + 32 * G * E + 512, [[1024, 34], [1, 512]],
                                 zpad[0:34, 0:512])
                        if b == 0:
                            for Ncol in (0, 33):
                                zdma(base + Ncol * E, [[G * E, 34], [1, 1024]],
                                     zpad[0:34, :])
                        else:
                            zdma(base + 33 * E, [[G * E, 34], [1, 1024]],
                                 zpad[0:34, :])
                            zdma(base + 0 * E,
                                 [[G * E, 34], [512, 2], [1, 256]],
                                 zpad[0:34, 0:512].rearrange(
                                     "p (u v) -> p u v", u=2))
                            zdma(base + 32 * E + 256,
                                 [[G * E, 34], [512, 2], [1, 256]],
                                 zpad[0:34, 0:512].rearrange(
                                     "p (u v) -> p u v", u=2))
                with tc.tile_pool(name=f"xp{img}", bufs=1) as xp, \
                     tc.tile_pool(name=f"xps{img}", bufs=4, space="PSUM") as xpp, \
                     tc.tile_pool(name=f"xst{img}", bufs=6) as xstp:
                    x_t = xp.tile([128, 2, HW], f32)
                    x16 = xp.tile([128, 2, HW], f16)
                    xv = x_d[img].rearrange("(cc p) q -> p cc q", cc=2, p=128)
                    for qc in range(4):
                        qs = slice(qc * 1024, (qc + 1) * 1024)
                        nc.sync.dma_start(x_t[:, :, qs], xv[:, :, qs])
                        nc.scalar.activation(x16[:, :, qs], x_t[:, :, qs],
                                             Act.Copy)
                    for qb in range(32):
                        st = xstp.tile([128, C], f16, tag="xst")
                        for cc in range(2):
                            ps = xpp.tile([128, 128], f16, tag="xps")
                            nc.tensor.transpose(
                                ps[:], x16[:, cc, qb * 128:(qb + 1) * 128], ident[:])
                            nc.scalar.activation(
                                st[:, cc * 128:(cc + 1) * 128], ps[:], Act.Copy)
                        nc.sync.dma_start(
                            xT_d[img, qb * 128:(qb + 1) * 128, :], st[:])

                # expansion: xT -> TC, 9 rectangular DRAM->DRAM DMAs.
                xsrc = xT_d[img].rearrange("q c -> (q c)")
                tdst = TC_ds[img][:].rearrange("t e -> (t e)")
                for a in range(2):
                    for b in range(2):
                        ab_base = ((a * 2 + b) * NT) * (4 * C)
                        yparts = ([(0, 32, 0, 0), (0, 32, 1, 1)] if a == 0
                                  else [(-1, 32, 0, 1), (0, 32, 1, 0)])
                        xparts = ([(0, 32, 0, None)] if b == 0
                                  else [(-1, 32, 0, 1), (0, 32, 1, 0)])
                        yhalves = [(m0 + h * 16, 16, y_base + h * 32, r)
                                   for (m0, mcnt, y_base, r) in yparts
                                   for h in range(2)]
                        for (m0, mcnt, y_base, r) in yhalves:
                            for (n0, ncnt, x_base, s) in xparts:
                                s0 = 0 if s is None else s
                                inner = 2 * C if s is None else C
                                src_off = (y_base * W + x_base) * C
                                dst_off = (ab_base
                                           + ((m0 + 1) * G + (n0 + 1)) * 4 * C
                                           + (r * 2 + s0) * C)
                                sdims = [[2 * W * C, mcnt], [2 * C, ncnt],
                                         [1, inner]]
                                ddims = [[G * 4 * C, mcnt], [4 * C, ncnt],
                                         [1, inner]]
                                src_ap = bass.AP(xsrc.tensor,
                                                 xsrc.offset + src_off, sdims)
                                dst_ap = bass.AP(tdst.tensor,
                                                 tdst.offset + dst_off, ddims)
                                nc.sync.dma_start(dst_ap, src_ap)

            # ---------- phase 3 (all images): taps ----------
            with tc.tile_pool(name="gp", bufs=5) as gpp, \
                 tc.tile_pool(name="zp", bufs=2) as zpp, \
                 tc.tile_pool(name="ztp", bufs=3) as ztp, \
                 tc.tile_pool(name="fp", bufs=2) as fpp, \
                 tc.tile_pool(name="pp", bufs=2, space="PSUM") as psp, \
                 tc.tile_pool(name="ac", bufs=1, space="PSUM") as accp, \
                 tc.tile_pool(name="cp", bufs=2, space="PSUM") as clsp:
                for img in range(n_img):
                    for pb in range(NPB):
                        acc = [accp.tile([128, 2, 512], f32, tag=f"acc{oc}", name=f"acc{oc}")
                               for oc in range(2)]
                        for k in range(9):
                            patch = gpp.tile([128, SL, 4 * C], f16, tag="patch")
                            for sl in range(SL):
                                gi = nc.gpsimd.indirect_dma_start(
                                    out=patch[:, sl, :], out_offset=None,
                                    in_=TC_ds[img][:],
                                    in_offset=bass.IndirectOffsetOnAxis(
                                        ap=idx16[:, img, k, pb, sl:sl + 1], axis=0),
                                    element_offset=0)
                                if queues and sl % 4:
                                    gi.ins.queue = "qPoolDynamic" + str(sl % 4)
                            zT = ztp.tile([128, 2, PB], f16, tag="zT")
                            for hf in range(4):
                                HS = SL // 4
                                psl = slice(hf * HS, (hf + 1) * HS)
                                m = zpp.tile([128, HS, 4, C], f16,
                                             tag=f"m{hf}", name="m")
                                wsl = w4[:, img, k, pb, psl]  # [128, HS, 4, 2]
                                wap = bass.AP(
                                    wsl.tensor, wsl.offset,
                                    [wsl.ap[0], [2, HS * 4], [0, C // 2], [1, 2]])
                                nc.vector.tensor_tensor(
                                    m[:].rearrange(
                                        "p s j (ch d) -> p (s j) ch d", d=2),
                                    patch[:, psl].rearrange(
                                        "p s (j ch d) -> p (s j) ch d",
                                        j=4, d=2),
                                    wap, Ao.mult)
                                a1 = zpp.tile([128, HS, C], f16,
                                              tag=f"a1{hf}", name="a1")
                                z = zpp.tile([128, HS, C], f16,
                                             tag=f"z{hf}", name="z")
                                nc.vector.tensor_tensor(
                                    a1[:], m[:, :, 0, :], m[:, :, 1, :], Ao.add)
                                nc.vector.tensor_tensor(
                                    z[:], m[:, :, 2, :], m[:, :, 3, :], Ao.add)
                                nc.vector.tensor_tensor(z[:], z[:], a1[:], Ao.add)

                                for cc in range(2):
                                    ps = psp.tile([128, 2, 128], f16, tag="pst")
                                    for j in range(2):
                                        nc.tensor.transpose(
                                            ps[:, j],
                                            z[:, j, cc * 128:(cc + 1) * 128],
                                            ident[:])
                                    nc.scalar.activation(
                                        zT[:, cc, hf * 256:(hf + 1) * 256],
                                        ps[:].rearrange("p a b -> p (a b)"),
                                        Act.Copy)
                                if hf % 2 == 1:
                                    nb = hf // 2
                                    for cc in range(2):
                                        for oc in range(2):
                                            nc.tensor.matmul(
                                                acc[oc][:, nb],
                                                wdef[:, k, cc, oc],
                                                zT[:, cc,
                                                   nb * 512:(nb + 1) * 512],
                                                start=(k == 0 and cc == 0),
                                                stop=(k == 8 and cc == 1))
                        feat = fpp.tile([128, 2, PB], f16, tag="feat")
                        for oc in range(2):
                            nc.scalar.activation(
                                feat[:, oc],
                                acc[oc][:].rearrange("p a b -> p (a b)"),
                                Act.Relu)
                        for half in range(2):
                            cps = clsp.tile([1, 512], f32, tag="cls")
                            for oc in range(2):
                                nc.tensor.matmul(
                                    cps[:], wcls[:, oc:oc + 1],
                                    feat[:, oc, half * 512:(half + 1) * 512],
                                    start=(oc == 0), stop=(oc == 1))
                            co = fpp.tile([1, 512], f32, tag="co")
                            nc.vector.tensor_tensor(
                                co[:], cps[:],
                                bcls[:].to_broadcast([1, 512]), Ao.add)
                            nc.scalar.dma_start(
                                out_d[img,
                                      pb * PB + half * 512:
                                      pb * PB + (half + 1) * 512]
                                .rearrange("a -> () a"),
                                co[:])
    nc.compile()
    return nc


def make_in_map(x_img, off_img, w_def, w_cls, b_cls, ident, wp, wcp):
    n_img = x_img.shape[0]
    idx, w4 = host_indices(off_img)
    return {
        "x": np.ascontiguousarray(x_img.reshape(n_img, C, HW).astype(np.float32)),
        "idx": idx,
        "w4": w4,
        "wdef": wp,
        "wcls": wcp,
        "bcls": np.asarray(b_cls, dtype=np.float32).reshape(1, 1),
        "ident": ident,
    }


_CACHE = {}


def _get_nc(n_img):
    if n_img not in _CACHE:
        _CACHE[n_img] = build(n_img)
    return _CACHE[n_img]


def kernel(x, offset, w_def, w_cls, b_cls):
    x = np.asarray(x, dtype=np.float32)
    offset = np.asarray(offset, dtype=np.float32)
    w_def = np.asarray(w_def, dtype=np.float32)
    w_cls = np.asarray(w_cls, dtype=np.float32)
    b_cls = np.asarray(b_cls, dtype=np.float32)
    N = x.shape[0]
    n_img = (N + N_CORES - 1) // N_CORES
    assert n_img * N_CORES == N, "batch must split evenly across 8 cores"

    ident = host_ident()
    wp, wcp = pack_weights(w_def, w_cls)
    nc = _get_nc(n_img)

    in_maps = []
    for cix in range(N_CORES):
        sl = slice(cix * n_img, (cix + 1) * n_img)
        in_maps.append(make_in_map(
            x[sl].reshape(n_img, C, HW), offset[sl],
            w_def, w_cls, b_cls, ident, wp, wcp))

    from concourse.bass_utils import run_bass_kernel_spmd
    res = run_bass_kernel_spmd(nc, in_maps, list(range(N_CORES)))
    outs = [res.results[cix]["out"].reshape(n_img, 1, H, W)
            for cix in range(N_CORES)]
    return np.concatenate(outs, axis=0).astype(np.float32)



# revision 11
# speedup vs baseline: 1.0525x; 1.0525x over previous
"""TRN2 Bass kernel v4 for nn_AlignHead (deformable conv 3x3 + ReLU + 1x1 cls).

Baseline (v1) structure -- 4-parity TC in DRAM + per-slot indirect gathers,
position-major interp on DVE, PE transpose-back, PSUM conv accumulation --
with the schedule fixed:
  - ALL bilinear index/weight math precomputed on the HOST from the offset
    input (v1 burned ~250us of serial DVE at kernel start computing it).
  - TC zero-pad DMAs moved from gpsimd (SWDGE) to sync (HWDGE): the Pool
    engine only issues the 576 indirect gathers.
  - Deeper tile buffering for run-ahead.
  - Interp / transpose-back / conv matmuls split into slot-halves (sl 0-3,
    sl 4-7 == matmul nb halves): each half starts as soon as its 4 gathers
    land instead of waiting for all 8, shortening the per-tap serial tail.
Numerics identical to v1: f16 gather/matmul data, f32 PSUM accumulation.
"""

import sys
sys.path.insert(0, '/opt/trn_rl_repo')
import numpy as np

import concourse.bass as bass
import concourse.tile as tile
from concourse import bacc, mybir

f32 = mybir.dt.float32
f16 = mybir.dt.float16
i32 = mybir.dt.int32
i16 = mybir.dt.int16

USE_DG = False  # gathers via dma_gather (1 call / (k,pb)) vs per-slot indirect

N_CORES = 8
C = 256
H = W = 64
HW = H * W
NPB = 4             # position blocks per image
PB = HW // NPB      # 1024 positions per block
SL = PB // 128      # 8 slots (gathers) per block
G = 34              # tile grid per axis (m in [-1,32] stored at m+1)
NT = G * G          # tiles per parity copy
TCN = 4 * NT        # tiles per image (4 parity copies)
Ao = mybir.AluOpType
Act = mybir.ActivationFunctionType


def host_ident():
    return np.eye(128, dtype=np.float16)


def pack_weights(w_def, w_cls):
    w = np.asarray(w_def, dtype=np.float32).reshape(256, 256, 9)
    wp = np.empty((128, 9, 2, 2, 128), dtype=np.float32)
    for cc in range(2):
        for oc in range(2):
            blk = w[oc * 128:(oc + 1) * 128, cc * 128:(cc + 1) * 128, :]
            wp[:, :, cc, oc, :] = blk.transpose(1, 2, 0)
    wc = np.asarray(w_cls, dtype=np.float32).reshape(256)
    wcp = wc.reshape(2, 128).T.copy()
    return wp, wcp


def host_indices(off_img):
    """offsets [n_img, HW, 18] f32 -> (idx [128, n_img, 9, 4, 8] i32 TC tile
    ids, w4 [128, n_img, 9, 4, 8, 4, 2] f16 corner weights, dup'd in pairs).

    Matches v1's on-device math: tile id = (2a+b)*NT + (m+1)*G + (n+1) with
    a = y0 mod 2, m = clip(floor(y0/2), -1, 32) (same for b, n from x0).
    Position mapping: pos = pb*1024 + sl*128 + p.
    """
    n_img = off_img.shape[0]
    off = np.asarray(off_img, dtype=np.float32)
    pos = np.arange(HW, dtype=np.float32)
    ybase = np.floor(pos / W)
    xbase = pos - ybase * W
    idx_out = np.zeros((128, n_img, 9, NPB, SL), dtype=np.int32)
    w4_out = np.zeros((128, n_img, 9, NPB, SL, 4, 2), dtype=np.float16)
    for k in range(9):
        ky, kx = k // 3, k % 3
        py = (ybase + np.float32(ky - 1)) + off[:, :, 2 * k]
        px = (xbase + np.float32(kx - 1)) + off[:, :, 2 * k + 1]
        y0 = np.floor(py)
        x0 = np.floor(px)
        fy = py - y0
        fx = px - x0
        wy0 = (1.0 - fy) * ((y0 >= 0) & (y0 <= H - 1))
        wy1 = fy * ((y0 >= -1) & (y0 <= H - 2))
        wx0 = (1.0 - fx) * ((x0 >= 0) & (x0 <= W - 1))
        wx1 = fx * ((x0 >= -1) & (x0 <= W - 2))
        ma = np.floor(y0 * 0.5)
        a = y0 - 2.0 * ma
        m = np.clip(ma, -1, 32)
        nb = np.floor(x0 * 0.5)
        b = x0 - 2.0 * nb
        n = np.clip(nb, -1, 32)
        tid = ((2 * a + b) * NT + (m + 1) * G + (n + 1)).astype(np.int32)
        wgt = np.stack([wy0 * wx0, wy0 * wx1, wy1 * wx0, wy1 * wx1],
                       axis=-1).astype(np.float16)          # [n, HW, 4]
        for img in range(n_img):
            for pb in range(NPB):
                sl = slice(pb * PB, (pb + 1) * PB)
                idx_out[:, img, k, pb, :] = tid[img, sl].reshape(SL, 128).T
                wb = wgt[img, sl].reshape(SL, 128, 4)       # [sl, p, j]
                for d in range(2):
                    w4_out[:, img, k, pb, :, :, d] = wb.transpose(1, 0, 2)
    return idx_out, w4_out


def build(n_img, queues=True):
    nc = bacc.Bacc("TRN2", target_bir_lowering=False, debug=False,
                   num_devices=N_CORES, num_swdge_queues=4 if queues else 1)
    x_d = nc.dram_tensor("x", [n_img, C, HW], f32, kind="ExternalInput").ap()
    idx_d = nc.dram_tensor("idx", [128, n_img, 9, NPB, SL], i32,
                           kind="ExternalInput").ap()
    if USE_DG:
        idxg_d = nc.dram_tensor("idxg", [128, n_img, 9, NPB, SL * 128 // 16],
                                i16, kind="ExternalInput").ap()
    w4_d = nc.dram_tensor("w4", [128, n_img, 9, NPB, SL, 4, 2], f16,
                          kind="ExternalInput").ap()
    wdef_d = nc.dram_tensor("wdef", [128, 9, 2, 2, 128], f32, kind="ExternalInput").ap()
    wcls_d = nc.dram_tensor("wcls", [128, 2], f32, kind="ExternalInput").ap()
    bcls_d = nc.dram_tensor("bcls", [1, 1], f32, kind="ExternalInput").ap()
    ident_d = nc.dram_tensor("ident", [128, 128], f16, kind="ExternalInput").ap()
    out_d = nc.dram_tensor("out", [n_img, HW], f32, kind="ExternalOutput").ap()

    xT_d = nc.dram_tensor("xT", [n_img, HW, C], f16).ap()
    TC_ds = [nc.dram_tensor(f"TC{i}", [TCN, 4 * C], f16).ap()
             for i in range(n_img)]

    with tile.TileContext(nc) as tc:
        with tc.tile_pool(name="const", bufs=1) as constp:
            ident = constp.tile([128, 128], f16)
            nc.sync.dma_start(ident[:], ident_d[:])
            wdef = constp.tile([128, 9, 2, 2, 128], f16)
            with tc.tile_pool(name="wstage", bufs=1) as wsp:
                wst = wsp.tile([128, 9, 2, 2, 128], f32)
                nc.sync.dma_start(wst[:], wdef_d[:])
                nc.scalar.activation(wdef[:], wst[:], Act.Copy)
            wcls = constp.tile([128, 2], f16)
            wclsf = constp.tile([128, 2], f32)
            nc.sync.dma_start(wclsf[:], wcls_d[:])
            nc.vector.tensor_copy(wcls[:], wclsf[:])
            bcls = constp.tile([1, 1], f32)
            nc.sync.dma_start(bcls[:], bcls_d[:])
            zpad = constp.tile([128, 1024], f16)
            nc.vector.memset(zpad[:], 0.0)
            idx16 = constp.tile([128, n_img, 9, NPB, SL], i32)
            nc.sync.dma_start(idx16[:], idx_d[:])
            if USE_DG:
                idxg = constp.tile([128, n_img, 9, NPB, SL * 128 // 16], i16)
                nc.sync.dma_start(idxg[:], idxg_d[:])
            w4 = constp.tile([128, n_img, 9, NPB, SL, 4, 2], f16)
            nc.sync.dma_start(w4[:], w4_d[:])

            for img in range(n_img):
                # ---------- phase 1: xT + TC build ----------
                tcflat = TC_ds[img][:].rearrange("t e -> (t e)")
                E = 4 * C

                def zdma(dst_off, dims, zsrc):
                    nc.sync.dma_start(
                        bass.AP(tcflat.tensor, tcflat.offset + dst_off, dims),
                        zsrc)

                for a in range(2):
                    for b in range(2):
                        base = ((a * 2 + b) * NT) * E
                        if a == 0:
                            for R in (0, 33):
                                zdma(base + R * G * E, [[1024, 34], [1, 1024]],
                                     zpad[0:34, :])
                        else:
                            zdma(base + 33 * G * E, [[1024, 34], [1, 1024]],
                                 zpad[0:34, :])
                            zdma(base + 0 * G * E, [[1024, 34], [1, 512]],
                                 zpad[0:34, 0:512])
                            zdma(base + 32 * G * E + 512, [[1024, 34], [1, 512]],
                                 zpad[0:34, 0:512])
                        if b == 0:
                            for Ncol in (0, 33):
                                zdma(base + Ncol * E, [[G * E, 34], [1, 1024]],
                                     zpad[0:34, :])
                        else:
                            zdma(base + 33 * E, [[G * E, 34], [1, 1024]],
                                 zpad[0:34, :])
                            zdma(base + 0 * E,
                                 [[G * E, 34], [512, 2], [1, 256]],
                                 zpad[0:34, 0:512].rearrange(
                                     "p (u v) -> p u v", u=2))
                            zdma(base + 32 * E + 256,
                                 [[G * E, 34], [512, 2], [1, 256]],
                                 zpad[0:34, 0:512].rearrange(
                                     "p (u v) -> p u v", u=2))
                with tc.tile_pool(name=f"xp{img}", bufs=1) as xp, \
                     tc.tile_pool(name=f"xps{img}", bufs=4, space="PSUM") as xpp, \
                     tc.tile_pool(name=f"xst{img}", bufs=6) as xstp:
                    x_t = xp.tile([128, 2, HW], f32)
                    x16 = xp.tile([128, 2, HW], f16)
                    xv = x_d[img].rearrange("(cc p) q -> p cc q", cc=2, p=128)
                    for qc in range(4):
                        qs = slice(qc * 1024, (qc + 1) * 1024)
                        nc.sync.dma_start(x_t[:, :, qs], xv[:, :, qs])
                        nc.scalar.activation(x16[:, :, qs], x_t[:, :, qs],
                                             Act.Copy)
                    for qb in range(32):
                        st = xstp.tile([128, C], f16, tag="xst")
                        for cc in range(2):
                            ps = xpp.tile([128, 128], f16, tag="xps")
                            nc.tensor.transpose(
                                ps[:], x16[:, cc, qb * 128:(qb + 1) * 128], ident[:])
                            nc.scalar.activation(
                                st[:, cc * 128:(cc + 1) * 128], ps[:], Act.Copy)
                        nc.sync.dma_start(
                            xT_d[img, qb * 128:(qb + 1) * 128, :], st[:])

                # expansion: xT -> TC, 9 rectangular DRAM->DRAM DMAs.
                xsrc = xT_d[img].rearrange("q c -> (q c)")
                tdst = TC_ds[img][:].rearrange("t e -> (t e)")
                for a in range(2):
                    for b in range(2):
                        ab_base = ((a * 2 + b) * NT) * (4 * C)
                        yparts = ([(0, 32, 0, 0), (0, 32, 1, 1)] if a == 0
                                  else [(-1, 32, 0, 1), (0, 32, 1, 0)])
                        xparts = ([(0, 32, 0, None)] if b == 0
                                  else [(-1, 32, 0, 1), (0, 32, 1, 0)])
                        yhalves = [(m0 + h * 16, 16, y_base + h * 32, r)
                                   for (m0, mcnt, y_base, r) in yparts
                                   for h in range(2)]
                        for (m0, mcnt, y_base, r) in yhalves:
                            for (n0, ncnt, x_base, s) in xparts:
                                s0 = 0 if s is None else s
                                inner = 2 * C if s is None else C
                                src_off = (y_base * W + x_base) * C
                                dst_off = (ab_base
                                           + ((m0 + 1) * G + (n0 + 1)) * 4 * C
                                           + (r * 2 + s0) * C)
                                sdims = [[2 * W * C, mcnt], [2 * C, ncnt],
                                         [1, inner]]
                                ddims = [[G * 4 * C, mcnt], [4 * C, ncnt],
                                         [1, inner]]
                                src_ap = bass.AP(xsrc.tensor,
                                                 xsrc.offset + src_off, sdims)
                                dst_ap = bass.AP(tdst.tensor,
                                                 tdst.offset + dst_off, ddims)
                                nc.sync.dma_start(dst_ap, src_ap)

            # ---------- phase 3 (all images): taps ----------
            with tc.tile_pool(name="gp", bufs=5) as gpp, \
                 tc.tile_pool(name="zp", bufs=2) as zpp, \
                 tc.tile_pool(name="ztp", bufs=3) as ztp, \
                 tc.tile_pool(name="fp", bufs=2) as fpp, \
                 tc.tile_pool(name="pp", bufs=2, space="PSUM") as psp, \
                 tc.tile_pool(name="ac", bufs=1, space="PSUM") as accp, \
                 tc.tile_pool(name="cp", bufs=2, space="PSUM") as clsp:
                for img in range(n_img):
                    for pb in range(NPB):
                        acc = [accp.tile([128, 2, 512], f32, tag=f"acc{oc}", name=f"acc{oc}")
                               for oc in range(2)]
                        for k in range(9):
                            patch = gpp.tile([128, SL, 4 * C], f16, tag="patch")
                            gcall = img * 36 + pb * 9 + k
                            if USE_DG:
                                # One dma_gather per (k, pb): 1024 tokens in a
                                # single SWDGE call (per-call desc-gen overhead
                                # was the gather bottleneck: 1.4us x 576).
                                # Token i lands at patch[i % 128, i // 128, :].
                                nc.gpsimd.dma_gather(
                                    out_ap=patch[:, :, :],
                                    in_ap=TC_ds[img][:],
                                    idxs_ap=idxg[:, img, k, pb, :],
                                    num_idxs=SL * 128,
                                    num_idxs_reg=SL * 128,
                                    elem_size=4 * C,
                                    queue_num=gcall % 4 if queues else 0,
                                )
                            else:
                                for sl in range(SL):
                                    gi = nc.gpsimd.indirect_dma_start(
                                        out=patch[:, sl, :], out_offset=None,
                                        in_=TC_ds[img][:],
                                        in_offset=bass.IndirectOffsetOnAxis(
                                            ap=idx16[:, img, k, pb, sl:sl + 1],
                                            axis=0),
                                        element_offset=0)
                                    if queues and sl % 4:
                                        gi.ins.queue = (
                                            "qPoolDynamic" + str(sl % 4))
                            zT = ztp.tile([128, 2, PB], f16, tag="zT")
                            for hf in range(4):
                                HS = SL // 4
                                psl = slice(hf * HS, (hf + 1) * HS)
                                m = zpp.tile([128, HS, 4, C], f16,
                                             tag=f"m{hf}", name="m")
                                wsl = w4[:, img, k, pb, psl]  # [128, HS, 4, 2]
                                wap = bass.AP(
                                    wsl.tensor, wsl.offset,
                                    [wsl.ap[0], [2, HS * 4], [0, C // 2], [1, 2]])
                                nc.vector.tensor_tensor(
                                    m[:].rearrange(
                                        "p s j (ch d) -> p (s j) ch d", d=2),
                                    patch[:, psl].rearrange(
                                        "p s (j ch d) -> p (s j) ch d",
                                        j=4, d=2),
                                    wap, Ao.mult)
                                a1 = zpp.tile([128, HS, C], f16,
                                              tag=f"a1{hf}", name="a1")
                                z = zpp.tile([128, HS, C], f16,
                                             tag=f"z{hf}", name="z")
                                nc.vector.tensor_tensor(
                                    a1[:], m[:, :, 0, :], m[:, :, 1, :], Ao.add)
                                nc.vector.tensor_tensor(
                                    z[:], m[:, :, 2, :], m[:, :, 3, :], Ao.add)
                                nc.vector.tensor_tensor(z[:], z[:], a1[:], Ao.add)

                                for cc in range(2):
                                    ps = psp.tile([128, 2, 128], f16, tag="pst")
                                    for j in range(2):
                                        nc.tensor.transpose(
                                            ps[:, j],
                                            z[:, j, cc * 128:(cc + 1) * 128],
                                            ident[:])
                                    nc.scalar.activation(
                                        zT[:, cc, hf * 256:(hf + 1) * 256],
                                        ps[:].rearrange("p a b -> p (a b)"),
                                        Act.Copy)
                                if hf % 2 == 1:
                                    nb = hf // 2
                                    for cc in range(2):
                                        for oc in range(2):
                                            nc.tensor.matmul(
                                                acc[oc][:, nb],
                                                wdef[:, k, cc, oc],
                                                zT[:, cc,
                                                   nb * 512:(nb + 1) * 512],
                                                start=(k == 0 and cc == 0),
                                                stop=(k == 8 and cc == 1))
                        feat = fpp.tile([128, 2, PB], f16, tag="feat")
                        for oc in range(2):
                            nc.scalar.activation(
                                feat[:, oc],
                                acc[oc][:].rearrange("p a b -> p (a b)"),
                                Act.Relu)
                        for half in range(2):
                            cps = clsp.tile([1, 512], f32, tag="cls")
                            for oc in range(2):
                                nc.tensor.matmul(
                                    cps[:], wcls[:, oc:oc + 1],
                                    feat[:, oc, half * 512:(half + 1) * 512],
                                    start=(oc == 0), stop=(oc == 1))
                            co = fpp.tile([1, 512], f32, tag="co")
                            nc.vector.tensor_tensor(
                                co[:], cps[:],
                                bcls[:].to_broadcast([1, 512]), Ao.add)
                            nc.scalar.dma_start(
                                out_d[img,
                                      pb * PB + half * 512:
                                      pb * PB + (half + 1) * 512]
                                .rearrange("a -> () a"),
                                co[:])
    nc.compile()
    return nc


def pack_gather_idx(idx):
    """idx [128, n, 9, NPB, SL] i32 -> dma_gather idxs layout
    [128, n, 9, NPB, SL*128//16] i16: gather-stream token i = sl*128 + p
    wraps to (partition i % 16, col i // 16); partitions 16..127 zero."""
    n = idx.shape[1]
    st = idx.transpose(1, 2, 3, 4, 0).reshape(n, 9, NPB, SL * 128)
    w = st.reshape(n, 9, NPB, SL * 128 // 16, 16).transpose(0, 1, 2, 4, 3)
    out = np.zeros((128, n, 9, NPB, SL * 128 // 16), dtype=np.int16)
    out[:16] = w.transpose(3, 0, 1, 2, 4)
    return out


def make_in_map(x_img, off_img, w_def, w_cls, b_cls, ident, wp, wcp):
    n_img = x_img.shape[0]
    idx, w4 = host_indices(off_img)
    return {
        "x": np.ascontiguousarray(x_img.reshape(n_img, C, HW).astype(np.float32)),
        "idx": idx,
        **({"idxg": pack_gather_idx(idx)} if USE_DG else {}),
        "w4": w4,
        "wdef": wp,
        "wcls": wcp,
        "bcls": np.asarray(b_cls, dtype=np.float32).reshape(1, 1),
        "ident": ident,
    }


_CACHE = {}


def _get_nc(n_img):
    if n_img not in _CACHE:
        _CACHE[n_img] = build(n_img)
    return _CACHE[n_img]


def kernel(x, offset, w_def, w_cls, b_cls):
    x = np.asarray(x, dtype=np.float32)
    offset = np.asarray(offset, dtype=np.float32)
    w_def = np.asarray(w_def, dtype=np.float32)
    w_cls = np.asarray(w_cls, dtype=np.float32)
    b_cls = np.asarray(b_cls, dtype=np.float32)
    N = x.shape[0]
    n_img = (N + N_CORES - 1) // N_CORES
    assert n_img * N_CORES == N, "batch must split evenly across 8 cores"

    ident = host_ident()
    wp, wcp = pack_weights(w_def, w_cls)
    nc = _get_nc(n_img)

    in_maps = []
    for cix in range(N_CORES):
        sl = slice(cix * n_img, (cix + 1) * n_img)
        in_maps.append(make_in_map(
            x[sl].reshape(n_img, C, HW), offset[sl],
            w_def, w_cls, b_cls, ident, wp, wcp))

    from concourse.bass_utils import run_bass_kernel_spmd
    res = run_bass_kernel_spmd(nc, in_maps, list(range(N_CORES)))
    outs = [res.results[cix]["out"].reshape(n_img, 1, H, W)
            for cix in range(N_CORES)]
    return np.concatenate(outs, axis=0).astype(np.float32)

